# revision 1
# baseline (speedup 1.0000x reference)
"""Trainium2 Bass kernel for a 2-layer GCN (GCNConv -> ReLU -> GCNConv -> sigmoid head).

Strategy (8 NeuronCores):
  - Node sharding: core c owns nodes [c*12500, (c+1)*12500), padded to 12544 = 98*128.
  - Edges are assigned to the core that owns their dst node (so segment-sums are local).
  - Using GCN algebra:  agg[i] = dis[i] * sum_{e: dst=i} (dis*h)[src_e] + (1/deg_i)*h[i] + b
    so the per-edge norm multiply disappears; rows are pre-scaled by dis once per node.
  - Per layer: local dense matmul h = x@W, scale by dis, AllGather a bf16 feature table
    (rows padded to 128 cols = 256B so dma_gather's elem-size constraint holds), then for
    each (src-chunk, dst-tile) cell gather the needed source rows with dma_gather (int16
    chunk-relative indices) and segment-sum via one-hot matmuls on the tensor engine.
"""

import numpy as np
import ml_dtypes

P = 128


class Cfg:
    def __init__(self, n_nodes, n_loc_real, nt, in_c, hid, nchunk, group_tiles=8):
        self.C = 8
        self.N = n_nodes
        self.NLOC_REAL = n_loc_real           # real nodes per core
        self.NT = nt                          # node tiles per core
        self.NLOC = nt * P                    # padded nodes per core
        self.NTAB = self.C * self.NLOC        # global table rows
        self.IN_C = in_c
        self.HID = hid
        self.TABW = 128                       # table row width (bf16) -> 256B rows
        self.NCHUNK = nchunk
        self.CR = self.NTAB // nchunk         # chunk rows (must be < 32768)
        assert self.CR * nchunk == self.NTAB and self.CR < 32768
        # tile groups for gathers: (tile_start, ntiles)
        self.groups = []
        t = 0
        while t < nt:
            g = min(group_tiles, nt - t)
            self.groups.append((t, g))
            t += g


def full_cfg():
    return Cfg(n_nodes=100000, n_loc_real=12500, nt=98, in_c=128, hid=64, nchunk=4,
               group_tiles=1)


def _prep(cfg, x, edge_index, W1, b1, W2, b2, Wl, bl):
    """Host-side sharding/partitioning. Returns (in_maps, B)."""
    C, NT, NLOC, NLOC_REAL = cfg.C, cfg.NT, cfg.NLOC, cfg.NLOC_REAL
    src = np.asarray(edge_index[0], dtype=np.int64)
    dst = np.asarray(edge_index[1], dtype=np.int64)
    core = dst // NLOC_REAL
    dst_local = (dst - core * NLOC_REAL).astype(np.int64)
    # global table row id (cores are padded to NLOC rows each)
    src_adj = (src // NLOC_REAL) * NLOC + (src % NLOC_REAL)
    chunk = src_adj // cfg.CR
    tile = dst_local // P

    ncell = cfg.NCHUNK * NT
    cell = (core * ncell + chunk * NT + tile).astype(np.int64)
    counts = np.bincount(cell, minlength=C * ncell)
    B = max(1, int(np.ceil(counts.max() / P)))
    S = B * P                                   # slots per cell
    NSLOT = ncell * S                           # per core
    NBLK = ncell * B

    order = np.argsort(cell, kind="stable")
    cell_s = cell[order]
    cell_start = np.zeros(C * ncell + 1, dtype=np.int64)
    np.cumsum(counts, out=cell_start[1:])
    rank = np.arange(len(src)) - cell_start[cell_s]
    slot_global = (cell_s % ncell) * S + rank + (cell_s // ncell) * NSLOT

    idx16 = np.zeros(C * NSLOT, dtype=np.int16)
    dstrel = np.full(C * NSLOT, -1.0, dtype=np.float32)
    idx16[slot_global] = (src_adj[order] - chunk[order] * cfg.CR).astype(np.int16)
    dstrel[slot_global] = (dst_local[order] - tile[order] * P).astype(np.float32)

    in_maps = []
    for c in range(C):
        m = {}
        xl = np.zeros((P, NLOC), dtype=np.float32)
        xl[:, :NLOC_REAL] = np.asarray(x[c * NLOC_REAL:(c + 1) * NLOC_REAL], np.float32).T
        m["xT"] = np.ascontiguousarray(xl)

        cnt = np.bincount(dst_local[core == c], minlength=NLOC).astype(np.int64)
        rowptr = np.concatenate([[0], np.cumsum(cnt)])
        m["rp0"] = np.ascontiguousarray(rowptr[:-1].reshape(NT, P).T.astype(np.int32))
        m["rp1"] = np.ascontiguousarray(rowptr[1:].reshape(NT, P).T.astype(np.int32))

        # gather indices, wrapped per call: call order = (chunk, group); within a call
        # logical index i -> partition i%16, col i//16; replicated to 8 groups of 16 parts
        base = c * NSLOT
        cols = []
        for ch in range(cfg.NCHUNK):
            for (t0, g) in cfg.groups:
                seg = idx16[base + (ch * NT + t0) * S: base + (ch * NT + t0 + g) * S]
                w = seg.reshape(-1, 16).T
                cols.append(np.tile(w, (8, 1)))
        m["gidx"] = np.ascontiguousarray(np.concatenate(cols, axis=1))

        dr = dstrel[base: base + NSLOT].reshape(NBLK, P).T
        m["dstrel"] = np.ascontiguousarray(dr.astype(np.float32))

        m["identm"] = np.eye(P, dtype=np.float32)
        m["iota"] = np.ascontiguousarray(
            np.tile(np.arange(P, dtype=np.float32), (P, 1)).astype(ml_dtypes.bfloat16))
        m["W1"] = np.asarray(W1, np.float32)
        m["W2"] = np.asarray(W2, np.float32)
        m["b1b"] = np.ascontiguousarray(np.tile(np.asarray(b1, np.float32)[None, :], (P, 1)))
        m["b2b"] = np.ascontiguousarray(np.tile(np.asarray(b2, np.float32)[None, :], (P, 1)))
        m["Wlb"] = np.ascontiguousarray(np.tile(np.asarray(Wl, np.float32)[:, 0][None, :], (P, 1)))
        in_maps.append(m)
    return in_maps, B


def _program(cfg, B, bl_value, no_gather=False, linearize=False):
    from concourse import bass, bacc, mybir
    import concourse.tile as tile

    f32 = mybir.dt.float32
    bf16 = mybir.dt.bfloat16
    i32 = mybir.dt.int32
    i16 = mybir.dt.int16
    AF = mybir.ActivationFunctionType
    OP = mybir.AluOpType

    NT, NLOC, HID, TABW, CR = cfg.NT, cfg.NLOC, cfg.HID, cfg.TABW, cfg.CR
    S = B * P
    NBLK = cfg.NCHUNK * NT * B
    NCOL = cfg.NCHUNK * NT * S // 16
    groups = cfg.groups
    rg = [list(range(cfg.C))]

    nc = bacc.Bacc("TRN2", target_bir_lowering=False, debug=False,
                   num_devices=cfg.C)
    xT_d = nc.dram_tensor("xT", [P, NLOC], f32, kind="ExternalInput")
    rp0_d = nc.dram_tensor("rp0", [P, NT], i32, kind="ExternalInput")
    rp1_d = nc.dram_tensor("rp1", [P, NT], i32, kind="ExternalInput")
    gidx_d = nc.dram_tensor("gidx", [P, NCOL], i16, kind="ExternalInput")
    dstrel_d = nc.dram_tensor("dstrel", [P, NBLK], f32, kind="ExternalInput")
    iota_d = nc.dram_tensor("iota", [P, P], bf16, kind="ExternalInput")
    identm_d = nc.dram_tensor("identm", [P, P], f32, kind="ExternalInput")
    W1_d = nc.dram_tensor("W1", [cfg.IN_C, HID], f32, kind="ExternalInput")
    W2_d = nc.dram_tensor("W2", [HID, HID], f32, kind="ExternalInput")
    b1b_d = nc.dram_tensor("b1b", [P, HID], f32, kind="ExternalInput")
    b2b_d = nc.dram_tensor("b2b", [P, HID], f32, kind="ExternalInput")
    Wlb_d = nc.dram_tensor("Wlb", [P, HID], f32, kind="ExternalInput")
    out_d = nc.dram_tensor("out", [NT, P], f32, kind="ExternalOutput")

    hloc = [nc.dram_tensor(f"h{l}loc", [NLOC, TABW], bf16) for l in (1, 2)]
    tabs = [nc.dram_tensor(f"tab{l}", [cfg.NTAB, TABW], bf16, addr_space="Shared")
            for l in (1, 2)]

    with tile.TileContext(nc, linearize=linearize) as tc:
        from contextlib import ExitStack
        with ExitStack() as ctx:
            const = ctx.enter_context(tc.tile_pool(name="const", bufs=1))
            persist = ctx.enter_context(tc.tile_pool(name="persist", bufs=1))
            tmp = ctx.enter_context(tc.tile_pool(name="tmp", bufs=3))
            psum = ctx.enter_context(tc.tile_pool(name="psum", bufs=2, space="PSUM"))

            nreg = {}
            ident = const.tile([P, P], f32, tag="ident")
            nc.sync.dma_start(out=ident[:], in_=identm_d[:, :])
            iota_t = const.tile([P, P], bf16, tag="iota")
            nc.sync.dma_start(out=iota_t[:], in_=iota_d[:, :])
            W1_t = const.tile([cfg.IN_C, HID], f32, tag="W1")
            nc.sync.dma_start(out=W1_t[:], in_=W1_d[:, :])
            W2_t = const.tile([HID, HID], f32, tag="W2")
            nc.sync.dma_start(out=W2_t[:], in_=W2_d[:, :])
            b1_t = const.tile([P, HID], f32, tag="b1")
            nc.sync.dma_start(out=b1_t[:], in_=b1b_d[:, :])
            b2_t = const.tile([P, HID], f32, tag="b2")
            nc.sync.dma_start(out=b2_t[:], in_=b2b_d[:, :])
            Wl_t = const.tile([P, HID], f32, tag="Wl")
            nc.sync.dma_start(out=Wl_t[:], in_=Wlb_d[:, :])
            bl_t = const.tile([P, 1], f32, tag="bl")
            nc.vector.memset(bl_t[:], float(bl_value))
            dstrel_t = const.tile([P, NBLK], f32, tag="dstrel")
            nc.sync.dma_start(out=dstrel_t[:], in_=dstrel_d[:, :])

            # deg -> dis = sqrt(1/deg), selfw = 1/deg
            rp0_t = const.tile([P, NT], i32, tag="rp0")
            nc.sync.dma_start(out=rp0_t[:], in_=rp0_d[:, :])
            rp1_t = const.tile([P, NT], i32, tag="rp1")
            nc.sync.dma_start(out=rp1_t[:], in_=rp1_d[:, :])
            degi = const.tile([P, NT], i32, tag="degi")
            nc.vector.tensor_tensor(out=degi[:], in0=rp1_t[:], in1=rp0_t[:], op=OP.subtract)
            degf = const.tile([P, NT], f32, tag="degf")
            nc.vector.tensor_copy(degf[:], degi[:])
            deg = const.tile([P, NT], f32, tag="deg")
            nc.vector.tensor_scalar(out=deg[:], in0=degf[:], scalar1=1.0, scalar2=None,
                                    op0=OP.add)
            selfw = const.tile([P, NT], f32, tag="selfw")
            nc.vector.reciprocal(out=selfw[:], in_=deg[:])
            dis = const.tile([P, NT], f32, tag="dis")
            nc.scalar.activation(out=dis[:], in_=selfw[:], func=AF.Sqrt)

            h_sb = persist.tile([P, NT * HID], f32, tag="h_sb")
            hp_sb = persist.tile([P, NT * TABW], bf16, tag="hp_sb")
            acc_sb = persist.tile([P, NT * HID], f32, tag="acc_sb")
            zT_sb = persist.tile([HID, NT * P], f32, tag="zT_sb")
            y_sb = persist.tile([P, NT], f32, tag="y_sb")
            # zero the bf16 table pad columns once (cols HID..TABW of each tile row)
            nc.vector.memset(hp_sb[:], 0.0)

            def layer_A(l, xT_t):
                """h = in @ W; h' = dis*h (bf16, into hp_sb); DMA h' to hloc[l-1]."""
                W_t = W1_t if l == 1 else W2_t
                for t in range(NT):
                    ps = psum.tile([P, HID], f32, tag="psA")
                    if l == 1:
                        lhsT = xT_t[:, t * P:(t + 1) * P]
                    else:
                        lhsT = zT_sb[:, t * P:(t + 1) * P]
                    nc.tensor.matmul(out=ps[:], lhsT=lhsT, rhs=W_t[:], start=True, stop=True)
                    nc.scalar.copy(out=h_sb[:, t * HID:(t + 1) * HID], in_=ps[:])
                    nc.scalar.activation(out=hp_sb[:, t * TABW:t * TABW + HID], in_=ps[:],
                                         func=AF.Copy, scale=dis[:, t:t + 1])
                for t in range(NT):
                    nc.sync.dma_start(out=hloc[l - 1][t * P:(t + 1) * P, :],
                                      in_=hp_sb[:, t * TABW:(t + 1) * TABW])

            def layer_agg(l):
                """AllGather table, gather+segment-sum into acc_sb."""
                nc.gpsimd.collective_compute(
                    "AllGather", mybir.AluOpType.bypass, replica_groups=rg,
                    ins=[hloc[l - 1][:, :]], outs=[tabs[l - 1][:, :]])
                with tc.tile_pool(name=f"gath{l}", bufs=2) as gp, \
                     tc.tile_pool(name=f"gidx{l}", bufs=3) as gip:
                    col = 0
                    blk = 0
                    for ch in range(cfg.NCHUNK):
                        for (t0, g) in groups:
                            nI = g * S
                            gi = gip.tile([P, nI // 16], i16, tag="gi")
                            nc.sync.dma_start(out=gi[:], in_=gidx_d[:, col:col + nI // 16])
                            col += nI // 16
                            gf = gp.tile([P, g * B, TABW], bf16, tag="gf")
                            if no_gather:
                                nc.vector.memset(gf[:], 0.0)
                            else:
                                nc.gpsimd.dma_gather(
                                    out_ap=gf[:], in_ap=tabs[l - 1][ch * CR:(ch + 1) * CR, :],
                                    idxs_ap=gi[:], num_idxs=nI,
                                    num_idxs_reg=nreg.setdefault(g, nc.gpsimd.to_reg(g * S)),
                                    elem_size=TABW)
                            ps = psum.tile([P, g * HID], f32, tag="psC")
                            for ti in range(g):
                                for b in range(B):
                                    oh = tmp.tile([P, P], bf16, tag="oh")
                                    nc.vector.tensor_scalar(
                                        out=oh[:], in0=iota_t[:],
                                        scalar1=dstrel_t[:, blk:blk + 1], scalar2=None,
                                        op0=OP.is_equal)
                                    nc.tensor.matmul(
                                        out=ps[:, ti * HID:(ti + 1) * HID],
                                        lhsT=oh[:],
                                        rhs=gf[:, ti * B + b, 0:HID],
                                        start=(b == 0), stop=(b == B - 1))
                                    blk += 1
                            dstslice = acc_sb[:, t0 * HID:(t0 + g) * HID]
                            if ch == 0:
                                nc.scalar.copy(out=dstslice, in_=ps[:])
                            else:
                                nc.vector.tensor_tensor(out=dstslice, in0=dstslice,
                                                        in1=ps[:], op=OP.add)

            def layer_post(l):
                """agg = dis*s + selfw*h + b; l1: relu+transpose into zT; l2: head."""
                b_t = b1_t if l == 1 else b2_t
                for t in range(NT):
                    t1 = tmp.tile([P, HID], f32, tag="t1")
                    nc.scalar.activation(out=t1[:], in_=acc_sb[:, t * HID:(t + 1) * HID],
                                         func=AF.Copy, scale=dis[:, t:t + 1])
                    t2 = tmp.tile([P, HID], f32, tag="t2")
                    nc.scalar.activation(out=t2[:], in_=h_sb[:, t * HID:(t + 1) * HID],
                                         func=AF.Copy, scale=selfw[:, t:t + 1])
                    nc.vector.tensor_tensor(out=t1[:], in0=t1[:], in1=t2[:], op=OP.add)
                    nc.vector.tensor_tensor(out=t1[:], in0=t1[:], in1=b_t[:], op=OP.add)
                    if l == 1:
                        z = tmp.tile([P, HID], f32, tag="z")
                        nc.scalar.activation(out=z[:], in_=t1[:], func=AF.Relu)
                        psE = psum.tile([HID, P], f32, tag="psE")
                        nc.tensor.transpose(out=psE[:], in_=z[:], identity=ident[:])
                        nc.scalar.copy(
                            out=zT_sb[:, t * P:(t + 1) * P], in_=psE[:])
                    else:
                        m = tmp.tile([P, HID], f32, tag="m")
                        nc.vector.tensor_tensor(out=m[:], in0=t1[:], in1=Wl_t[:], op=OP.mult)
                        r = tmp.tile([P, 1], f32, tag="r")
                        nc.vector.tensor_reduce(out=r[:], in_=m[:],
                                                axis=mybir.AxisListType.X, op=OP.add)
                        nc.scalar.activation(out=y_sb[:, t:t + 1], in_=r[:],
                                             func=AF.Sigmoid, bias=bl_t[:, 0:1])

            with tc.tile_pool(name="xt", bufs=1) as xtp:
                xT_t = xtp.tile([P, NLOC], f32, tag="xT")
                nc.sync.dma_start(out=xT_t[:], in_=xT_d[:, :])
                layer_A(1, xT_t)
            layer_agg(1)
            layer_post(1)
            layer_A(2, None)
            layer_agg(2)
            layer_post(2)

            psG = psum.tile([NT, P], f32, tag="psG")
            nc.tensor.matmul(out=psG[:], lhsT=y_sb[:, :NT], rhs=ident[:],
                             start=True, stop=True, is_transpose=True)
            og = tmp.tile([NT, P], f32, tag="og")
            nc.scalar.copy(out=og[:], in_=psG[:])
            nc.sync.dma_start(out=out_d[:, :], in_=og[:])
    nc.compile()
    return nc


def kernel(x, edge_index, W1, b1, W2, b2, Wl, bl):
    from concourse.bass_utils import run_bass_kernel_spmd
    cfg = full_cfg()
    in_maps, B = _prep(cfg, x, edge_index, W1, b1, W2, b2, Wl, bl)
    nc = _program(cfg, B, float(np.asarray(bl).reshape(-1)[0]))
    res = run_bass_kernel_spmd(nc, in_maps, list(range(cfg.C)))
    outs = []
    for c in range(cfg.C):
        o = np.asarray(res.results[c]["out"], dtype=np.float32).reshape(cfg.NLOC)
        outs.append(o[:cfg.NLOC_REAL])
    return np.concatenate(outs).reshape(cfg.N, 1).astype(np.float32)



# revision 9
# speedup vs baseline: 1.0420x; 1.0420x over previous
"""Trainium2 Bass kernel for a 2-layer GCN (GCNConv -> ReLU -> GCNConv -> sigmoid head).

Strategy (8 NeuronCores):
  - Node sharding: core c owns nodes [c*12500, (c+1)*12500), padded to 12544 = 98*128.
  - Edges are assigned to the core that owns their dst node (so segment-sums are local).
  - Using GCN algebra:  agg[i] = dis[i] * sum_{e: dst=i} (dis*h)[src_e] + (1/deg_i)*h[i] + b
    so the per-edge norm multiply disappears; rows are pre-scaled by dis once per node.
  - Per layer: local dense matmul h = x@W, scale by dis, AllGather a bf16 feature table
    (rows padded to 128 cols = 256B so dma_gather's elem-size constraint holds), then for
    each (src-chunk, dst-tile) cell gather the needed source rows with dma_gather (int16
    chunk-relative indices) and segment-sum via one-hot matmuls on the tensor engine.
"""

import numpy as np
import ml_dtypes

P = 128


class Cfg:
    def __init__(self, n_nodes, n_loc_real, nt, in_c, hid, nchunk, group_tiles=8):
        self.C = 8
        self.N = n_nodes
        self.NLOC_REAL = n_loc_real           # real nodes per core
        self.NT = nt                          # node tiles per core
        self.NLOC = nt * P                    # padded nodes per core
        self.NTAB = self.C * self.NLOC        # global table rows
        self.IN_C = in_c
        self.HID = hid
        self.TABW = 128                       # table row width (bf16) -> 256B rows
        self.NCHUNK = nchunk
        self.CR = self.NTAB // nchunk         # chunk rows (must be < 32768)
        assert self.CR * nchunk == self.NTAB and self.CR < 32768
        # tile groups for gathers: (tile_start, ntiles)
        self.groups = []
        t = 0
        while t < nt:
            g = min(group_tiles, nt - t)
            self.groups.append((t, g))
            t += g
        # run-gather: blocks per gather call (RUNB*128 idxs <= 1024 HW limit)
        self.RUNB = 7


def full_cfg():
    return Cfg(n_nodes=100000, n_loc_real=12500, nt=98, in_c=128, hid=64, nchunk=4,
               group_tiles=2)


def _prep(cfg, x, edge_index, W1, b1, W2, b2, Wl, bl):
    """Host-side sharding/partitioning. Returns (in_maps, B)."""
    C, NT, NLOC, NLOC_REAL = cfg.C, cfg.NT, cfg.NLOC, cfg.NLOC_REAL
    src = np.asarray(edge_index[0], dtype=np.int64)
    dst = np.asarray(edge_index[1], dtype=np.int64)
    core = dst // NLOC_REAL
    dst_local = (dst - core * NLOC_REAL).astype(np.int64)
    # global table row id (cores are padded to NLOC rows each)
    src_adj = (src // NLOC_REAL) * NLOC + (src % NLOC_REAL)
    chunk = src_adj // cfg.CR
    tile = dst_local // P

    ncell = cfg.NCHUNK * NT
    cell = (core * ncell + chunk * NT + tile).astype(np.int64)
    counts = np.bincount(cell, minlength=C * ncell)
    B = max(1, int(np.ceil(counts.max() / P)))
    S = B * P                                   # slots per cell
    NSLOT = ncell * S                           # per core
    NBLK = ncell * B

    order = np.argsort(cell, kind="stable")
    cell_s = cell[order]
    cell_start = np.zeros(C * ncell + 1, dtype=np.int64)
    np.cumsum(counts, out=cell_start[1:])
    rank = np.arange(len(src)) - cell_start[cell_s]
    slot_global = (cell_s % ncell) * S + rank + (cell_s // ncell) * NSLOT

    idx16 = np.zeros(C * NSLOT, dtype=np.int16)
    dstrel = np.full(C * NSLOT, -1.0, dtype=np.float32)
    idx16[slot_global] = (src_adj[order] - chunk[order] * cfg.CR).astype(np.int16)
    dstrel[slot_global] = (dst_local[order] - tile[order] * P).astype(np.float32)

    in_maps = []
    for c in range(C):
        m = {}
        xl = np.zeros((P, NLOC), dtype=np.float32)
        xl[:, :NLOC_REAL] = np.asarray(x[c * NLOC_REAL:(c + 1) * NLOC_REAL], np.float32).T
        m["xT"] = np.ascontiguousarray(xl)

        cnt = np.bincount(dst_local[core == c], minlength=NLOC).astype(np.int64)
        rowptr = np.concatenate([[0], np.cumsum(cnt)])
        m["rp0"] = np.ascontiguousarray(rowptr[:-1].reshape(NT, P).T.astype(np.int32))
        m["rp1"] = np.ascontiguousarray(rowptr[1:].reshape(NT, P).T.astype(np.int32))

        # gather indices, wrapped per call: call order = (chunk, run); a run covers
        # RUNB consecutive 128-slot blocks of the chunk's cell-major slot stream.
        # Within a call: logical index i -> partition i%16, col i//16; replicated
        # to 8 groups of 16 partitions.
        base = c * NSLOT
        RUN = cfg.RUNB * P
        chunk_slots = NT * S
        nrun = -(-chunk_slots // RUN)
        cols = []
        for ch in range(cfg.NCHUNK):
            cbase = base + ch * chunk_slots
            for r in range(nrun):
                lo = cbase + r * RUN
                hi = min(lo + RUN, cbase + chunk_slots)
                seg = idx16[lo:hi]
                if len(seg) < RUN:
                    seg = np.concatenate([seg, np.zeros(RUN - len(seg), np.int16)])
                w = seg.reshape(-1, 16).T
                cols.append(np.tile(w, (8, 1)))
        m["gidx"] = np.ascontiguousarray(np.concatenate(cols, axis=1))

        dr = dstrel[base: base + NSLOT].reshape(NBLK, P).T
        m["dstrel"] = np.ascontiguousarray(dr.astype(np.float32))

        m["identm"] = np.eye(P, dtype=np.float32)
        m["iota"] = np.ascontiguousarray(
            np.tile(np.arange(P, dtype=np.float32), (P, 1)).astype(ml_dtypes.bfloat16))
        m["W1"] = np.asarray(W1, np.float32)
        m["W2"] = np.asarray(W2, np.float32)
        m["b1b"] = np.ascontiguousarray(np.tile(np.asarray(b1, np.float32)[None, :], (P, 1)))
        m["b2b"] = np.ascontiguousarray(np.tile(np.asarray(b2, np.float32)[None, :], (P, 1)))
        m["Wlb"] = np.ascontiguousarray(np.tile(np.asarray(Wl, np.float32)[:, 0][None, :], (P, 1)))
        in_maps.append(m)
    return in_maps, B


def _program(cfg, B, bl_value, no_gather=False, linearize=False):
    from concourse import bass, bacc, mybir
    import concourse.tile as tile

    f32 = mybir.dt.float32
    bf16 = mybir.dt.bfloat16
    i32 = mybir.dt.int32
    i16 = mybir.dt.int16
    AF = mybir.ActivationFunctionType
    OP = mybir.AluOpType

    NT, NLOC, HID, TABW, CR = cfg.NT, cfg.NLOC, cfg.HID, cfg.TABW, cfg.CR
    S = B * P
    NBLK = cfg.NCHUNK * NT * B
    RUNB = cfg.RUNB
    RUN = RUNB * P
    NRUN = -(-(NT * S) // RUN)          # gather calls per chunk
    NCOL = cfg.NCHUNK * NRUN * RUN // 16
    rg = [list(range(cfg.C))]

    nc = bacc.Bacc("TRN2", target_bir_lowering=False, debug=False,
                   num_devices=cfg.C)
    xT_d = nc.dram_tensor("xT", [P, NLOC], f32, kind="ExternalInput")
    rp0_d = nc.dram_tensor("rp0", [P, NT], i32, kind="ExternalInput")
    rp1_d = nc.dram_tensor("rp1", [P, NT], i32, kind="ExternalInput")
    gidx_d = nc.dram_tensor("gidx", [P, NCOL], i16, kind="ExternalInput")
    dstrel_d = nc.dram_tensor("dstrel", [P, NBLK], f32, kind="ExternalInput")
    iota_d = nc.dram_tensor("iota", [P, P], bf16, kind="ExternalInput")
    identm_d = nc.dram_tensor("identm", [P, P], f32, kind="ExternalInput")
    W1_d = nc.dram_tensor("W1", [cfg.IN_C, HID], f32, kind="ExternalInput")
    W2_d = nc.dram_tensor("W2", [HID, HID], f32, kind="ExternalInput")
    b1b_d = nc.dram_tensor("b1b", [P, HID], f32, kind="ExternalInput")
    b2b_d = nc.dram_tensor("b2b", [P, HID], f32, kind="ExternalInput")
    Wlb_d = nc.dram_tensor("Wlb", [P, HID], f32, kind="ExternalInput")
    out_d = nc.dram_tensor("out", [NT, P], f32, kind="ExternalOutput")

    hloc = [nc.dram_tensor(f"h{l}loc", [NLOC, TABW], bf16) for l in (1, 2)]
    tabs = [nc.dram_tensor(f"tab{l}", [cfg.NTAB, TABW], bf16, addr_space="Shared")
            for l in (1, 2)]

    with tile.TileContext(nc, linearize=linearize) as tc:
        from contextlib import ExitStack
        with ExitStack() as ctx:
            const = ctx.enter_context(tc.tile_pool(name="const", bufs=1))
            persist = ctx.enter_context(tc.tile_pool(name="persist", bufs=1))
            tmp = ctx.enter_context(tc.tile_pool(name="tmp", bufs=3))
            psum = ctx.enter_context(tc.tile_pool(name="psum", bufs=2, space="PSUM"))

            nreg = {}
            ident = const.tile([P, P], f32, tag="ident")
            nc.sync.dma_start(out=ident[:], in_=identm_d[:, :])
            iota_t = const.tile([P, P], bf16, tag="iota")
            nc.sync.dma_start(out=iota_t[:], in_=iota_d[:, :])
            W1_t = const.tile([cfg.IN_C, HID], f32, tag="W1")
            nc.sync.dma_start(out=W1_t[:], in_=W1_d[:, :])
            W2_t = const.tile([HID, HID], f32, tag="W2")
            nc.sync.dma_start(out=W2_t[:], in_=W2_d[:, :])
            b1_t = const.tile([P, HID], f32, tag="b1")
            nc.sync.dma_start(out=b1_t[:], in_=b1b_d[:, :])
            b2_t = const.tile([P, HID], f32, tag="b2")
            nc.sync.dma_start(out=b2_t[:], in_=b2b_d[:, :])
            Wl_t = const.tile([P, HID], f32, tag="Wl")
            nc.sync.dma_start(out=Wl_t[:], in_=Wlb_d[:, :])
            bl_t = const.tile([P, 1], f32, tag="bl")
            nc.vector.memset(bl_t[:], float(bl_value))
            dstrel_t = const.tile([P, NBLK], f32, tag="dstrel")
            nc.sync.dma_start(out=dstrel_t[:], in_=dstrel_d[:, :])

            # deg -> dis = sqrt(1/deg), selfw = 1/deg
            rp0_t = const.tile([P, NT], i32, tag="rp0")
            nc.sync.dma_start(out=rp0_t[:], in_=rp0_d[:, :])
            rp1_t = const.tile([P, NT], i32, tag="rp1")
            nc.sync.dma_start(out=rp1_t[:], in_=rp1_d[:, :])
            degi = const.tile([P, NT], i32, tag="degi")
            nc.vector.tensor_tensor(out=degi[:], in0=rp1_t[:], in1=rp0_t[:], op=OP.subtract)
            degf = const.tile([P, NT], f32, tag="degf")
            nc.vector.tensor_copy(degf[:], degi[:])
            deg = const.tile([P, NT], f32, tag="deg")
            nc.vector.tensor_scalar(out=deg[:], in0=degf[:], scalar1=1.0, scalar2=None,
                                    op0=OP.add)
            selfw = const.tile([P, NT], f32, tag="selfw")
            nc.vector.reciprocal(out=selfw[:], in_=deg[:])
            dis = const.tile([P, NT], f32, tag="dis")
            nc.scalar.activation(out=dis[:], in_=selfw[:], func=AF.Sqrt)

            h_sb = persist.tile([P, NT * HID], f32, tag="h_sb")
            hp_sb = persist.tile([P, NT * TABW], bf16, tag="hp_sb")
            acc_sb = persist.tile([P, NT * HID], f32, tag="acc_sb")
            zT_sb = persist.tile([HID, NT * P], f32, tag="zT_sb")
            y_sb = persist.tile([P, NT], f32, tag="y_sb")
            # zero the bf16 table pad columns once (cols HID..TABW of each tile row)
            nc.vector.memset(hp_sb[:], 0.0)

            def layer_A(l, xT_t):
                """h = in @ W; h' = dis*h (bf16, into hp_sb); DMA h' to hloc[l-1]."""
                W_t = W1_t if l == 1 else W2_t
                for t in range(NT):
                    ps = psum.tile([P, HID], f32, tag="psA")
                    if l == 1:
                        lhsT = xT_t[:, t * P:(t + 1) * P]
                    else:
                        lhsT = zT_sb[:, t * P:(t + 1) * P]
                    nc.tensor.matmul(out=ps[:], lhsT=lhsT, rhs=W_t[:], start=True, stop=True)
                    nc.scalar.copy(out=h_sb[:, t * HID:(t + 1) * HID], in_=ps[:])
                    nc.scalar.activation(out=hp_sb[:, t * TABW:t * TABW + HID], in_=ps[:],
                                         func=AF.Copy, scale=dis[:, t:t + 1])
                for t in range(NT):
                    nc.sync.dma_start(out=hloc[l - 1][t * P:(t + 1) * P, :],
                                      in_=hp_sb[:, t * TABW:(t + 1) * TABW])

            def layer_agg(l):
                """AllGather table, run-gather (RUNB blocks/call) + segment-sum."""
                nc.gpsimd.collective_compute(
                    "AllGather", mybir.AluOpType.bypass, replica_groups=rg,
                    ins=[hloc[l - 1][:, :]], outs=[tabs[l - 1][:, :]])
                nblk_chunk = NT * B
                with tc.tile_pool(name=f"gath{l}", bufs=3) as gp, \
                     tc.tile_pool(name=f"gidx{l}", bufs=3) as gip:
                    for ch in range(cfg.NCHUNK):
                        ps = None
                        for r in range(NRUN):
                            col = (ch * NRUN + r) * RUN // 16
                            gi = gip.tile([P, RUN // 16], i16, tag="gi")
                            nc.sync.dma_start(out=gi[:], in_=gidx_d[:, col:col + RUN // 16])
                            gf = gp.tile([P, RUNB, TABW], bf16, tag="gf")
                            if no_gather:
                                nc.vector.memset(gf[:], 0.0)
                            else:
                                nc.gpsimd.dma_gather(
                                    out_ap=gf[:], in_ap=tabs[l - 1][ch * CR:(ch + 1) * CR, :],
                                    idxs_ap=gi[:], num_idxs=RUN,
                                    num_idxs_reg=nreg.setdefault(RUN, nc.gpsimd.to_reg(RUN)),
                                    elem_size=TABW)
                            for j in range(RUNB):
                                bg = r * RUNB + j
                                if bg >= nblk_chunk:
                                    break
                                t, b = bg // B, bg % B
                                if b == 0:
                                    ps = psum.tile([P, HID], f32, tag="psC")
                                oh = tmp.tile([P, P], bf16, tag="oh")
                                nc.vector.tensor_scalar(
                                    out=oh[:], in0=iota_t[:],
                                    scalar1=dstrel_t[:, ch * nblk_chunk + bg:
                                                     ch * nblk_chunk + bg + 1],
                                    scalar2=None, op0=OP.is_equal)
                                nc.tensor.matmul(
                                    out=ps[:], lhsT=oh[:], rhs=gf[:, j, 0:HID],
                                    start=(b == 0), stop=(b == B - 1))
                                if b == B - 1:
                                    dstslice = acc_sb[:, t * HID:(t + 1) * HID]
                                    if ch == 0:
                                        nc.scalar.copy(out=dstslice, in_=ps[:])
                                    else:
                                        nc.vector.tensor_tensor(
                                            out=dstslice, in0=dstslice,
                                            in1=ps[:], op=OP.add)

            def layer_post(l):
                """agg = dis*s + selfw*h + b; l1: relu+transpose into zT; l2: head."""
                b_t = b1_t if l == 1 else b2_t
                for t in range(NT):
                    t1 = tmp.tile([P, HID], f32, tag="t1")
                    nc.scalar.activation(out=t1[:], in_=acc_sb[:, t * HID:(t + 1) * HID],
                                         func=AF.Copy, scale=dis[:, t:t + 1])
                    t2 = tmp.tile([P, HID], f32, tag="t2")
                    nc.scalar.activation(out=t2[:], in_=h_sb[:, t * HID:(t + 1) * HID],
                                         func=AF.Copy, scale=selfw[:, t:t + 1])
                    nc.vector.tensor_tensor(out=t1[:], in0=t1[:], in1=t2[:], op=OP.add)
                    nc.vector.tensor_tensor(out=t1[:], in0=t1[:], in1=b_t[:], op=OP.add)
                    if l == 1:
                        z = tmp.tile([P, HID], f32, tag="z")
                        nc.scalar.activation(out=z[:], in_=t1[:], func=AF.Relu)
                        psE = psum.tile([HID, P], f32, tag="psE")
                        nc.tensor.transpose(out=psE[:], in_=z[:], identity=ident[:])
                        nc.scalar.copy(
                            out=zT_sb[:, t * P:(t + 1) * P], in_=psE[:])
                    else:
                        m = tmp.tile([P, HID], f32, tag="m")
                        nc.vector.tensor_tensor(out=m[:], in0=t1[:], in1=Wl_t[:], op=OP.mult)
                        r = tmp.tile([P, 1], f32, tag="r")
                        nc.vector.tensor_reduce(out=r[:], in_=m[:],
                                                axis=mybir.AxisListType.X, op=OP.add)
                        nc.scalar.activation(out=y_sb[:, t:t + 1], in_=r[:],
                                             func=AF.Sigmoid, bias=bl_t[:, 0:1])

            with tc.tile_pool(name="xt", bufs=1) as xtp:
                xT_t = xtp.tile([P, NLOC], f32, tag="xT")
                nc.sync.dma_start(out=xT_t[:], in_=xT_d[:, :])
                layer_A(1, xT_t)
            layer_agg(1)
            layer_post(1)
            layer_A(2, None)
            layer_agg(2)
            layer_post(2)

            psG = psum.tile([NT, P], f32, tag="psG")
            nc.tensor.matmul(out=psG[:], lhsT=y_sb[:, :NT], rhs=ident[:],
                             start=True, stop=True, is_transpose=True)
            og = tmp.tile([NT, P], f32, tag="og")
            nc.scalar.copy(out=og[:], in_=psG[:])
            nc.sync.dma_start(out=out_d[:, :], in_=og[:])
    nc.compile()
    return nc


def kernel(x, edge_index, W1, b1, W2, b2, Wl, bl):
    from concourse.bass_utils import run_bass_kernel_spmd
    cfg = full_cfg()
    in_maps, B = _prep(cfg, x, edge_index, W1, b1, W2, b2, Wl, bl)
    nc = _program(cfg, B, float(np.asarray(bl).reshape(-1)[0]))
    res = run_bass_kernel_spmd(nc, in_maps, list(range(cfg.C)))
    outs = []
    for c in range(cfg.C):
        o = np.asarray(res.results[c]["out"], dtype=np.float32).reshape(cfg.NLOC)
        outs.append(o[:cfg.NLOC_REAL])
    return np.concatenate(outs).reshape(cfg.N, 1).astype(np.float32)

